# revision 9
# baseline (speedup 1.0000x reference)
"""Instant-NGP hash-encoding forward on 8 TRN2 NeuronCores.

Data-parallel over points, processed in CHUNKS sequential launches per call
(pipelines device exec + host decode under the slow axon wire). Per core per
launch:
  - Each core receives 2 of the 16 level tables (fp16, padded to 16384
    entries); an AllGather over NeuronLink assembles the full table set in
    DRAM (cuts the host->device table upload 8x).
  - Per level the table is broadcast-DMA'd into SBUF across all 128
    partitions; gather via GPSIMD ap_gather; DVE computes cell coords,
    hash indices, trilinear weights, 8-corner weighted reduction.
  - Per-level results go to a DRAM scratch (level-major); a final pass
    interleaves them to [N, 32], quantizes to int8 with a per-point fp16
    scale (max |feat| over 32 features, reciprocal_approx_fast), and packs
    q[32]+scale[2B] into a 34-byte/point output row (single d2h tensor).

Wire-format rationale: the axon tunnel runs ~30-40MB/s half-duplex, so bytes
on the wire dominate wall time. coords ship as uint16 (floor(c*65536)),
output ships as int8+fp16-scale (34B/point); host decodes q * (scale/126).
The jitted shard_map executable is built ONCE and cached at module level.
"""
import numpy as np
from concurrent.futures import ThreadPoolExecutor

import concourse.bass as bass
import concourse.mybir as mybir
from concourse import bacc
from concourse.tile import TileContext

F32 = mybir.dt.float32
F16 = mybir.dt.float16
I32 = mybir.dt.int32
I16 = mybir.dt.int16
I8 = mybir.dt.int8
U16 = mybir.dt.uint16
AL = mybir.AluOpType
AX = mybir.AxisListType

NUM_LEVELS = 16
TABLE_SIZE = 2 ** 14
MIN_RES, MAX_RES = 16, 512
FEAT = 2
N_POINTS = 1 << 20
N_CORES = 8
PI1, PI2 = 2654435761, 805459861
P1L = PI1 & (TABLE_SIZE - 1)
P2L = PI2 & (TABLE_SIZE - 1)

_b = np.exp((np.log(MAX_RES) - np.log(MIN_RES)) / (NUM_LEVELS - 1))
RES = np.floor(MIN_RES * _b ** np.arange(NUM_LEVELS)).astype(np.int64)
COUNTS = np.minimum((RES + 1) ** 3, TABLE_SIZE)
OFFSETS = np.concatenate([[0], np.cumsum(COUNTS)])
DENSE = [int(COUNTS[l]) == int((RES[l] + 1) ** 3) for l in range(NUM_LEVELS)]

CHUNKS = 4                       # sequential launches per kernel() call
CHUNK_N = N_POINTS // CHUNKS     # global points per launch
NC_N = CHUNK_N // N_CORES        # 65536 points per core per launch
P = 128
PPP = NC_N // P                  # 512 points per partition
T = 64                           # points per partition per tile
NT = PPP // T                    # tiles per core
NI = 16 * T * 8                  # ap_gather num_idxs per 16-partition group
NE = TABLE_SIZE
LPC = NUM_LEVELS // N_CORES      # table levels shipped per core (allgathered)

QMAX = 126.0                     # int8 quant headroom (|q| <= 126)
OB = 34                          # output row bytes: 32 int8 feats + fp16 scale

# f32 blob slots (units of T elements)
S_SX, S_XF, S_GT = 0, 1, 2
S_FL = 3   # 3 slots
S_FR = 6   # 3 slots
S_W0 = 9   # 3 slots
S_WXY = 12  # 4 slots
S_PROD = 16  # 16 slots
S_CU = 32   # u16->f32 staging
NBF = 33
# i32 blob slots
S_XI = 0
S_FI = 1   # 3 slots
S_HX1 = 4
S_HY0, S_HY1, S_HZ0, S_HZ1 = 5, 6, 7, 8
S_TMP = 9
S_HXY = 10  # 4 slots
S_IDX = 14  # 8 slots
NBI = 22


def _ap(tile_ap, part_off, part_step, part_cnt, elem_off, dims):
    pitch = tile_ap.ap[0][0]
    return bass.AP(
        tile_ap.tensor,
        tile_ap.offset + part_off * pitch + elem_off,
        [[part_step * pitch, part_cnt]] + dims,
    )


def _build_nc():
    nc = bacc.Bacc("TRN2", target_bir_lowering=False, debug=False)
    coords = nc.dram_tensor("coords", [NC_N, 3], U16, kind="ExternalInput")
    emb_in = nc.dram_tensor("emb16", [LPC, NE * FEAT], F16, kind="ExternalInput")
    out = nc.dram_tensor("out", [NC_N, OB], I8, kind="ExternalOutput")
    out16 = out.bitcast(F16)      # [NC_N, OB//2]

    coords_v = coords[:, :].rearrange("(p q) c -> p (q c)", p=P)
    out_v = out[:, :].rearrange("(p q) g -> p (q g)", p=P)
    s_v = out16[:, :].rearrange("(p q) g -> p (q g)", p=P)

    with TileContext(nc) as tc:
        with tc.tile_pool(name="dram", bufs=1, space="DRAM") as dpool, \
             tc.tile_pool(name="tab", bufs=1) as tabp, \
             tc.tile_pool(name="coord", bufs=1) as cpool, \
             tc.tile_pool(name="gat", bufs=1) as gpool, \
             tc.tile_pool(name="blob", bufs=2) as bp, \
             tc.tile_pool(name="idxw", bufs=2) as xp, \
             tc.tile_pool(name="io", bufs=2) as iop:

            scratch = dpool.tile([NUM_LEVELS * NT * P, T * FEAT], F32)
            emb_stage = dpool.tile([LPC, NE * FEAT], F16)
            emb_full = dpool.tile([NUM_LEVELS, NE * FEAT], F16)

            nc.sync.dma_start(out=emb_stage[:, :], in_=emb_in[:, :])
            nc.gpsimd.collective_compute(
                kind="AllGather",
                op=AL.bypass,
                replica_groups=[list(range(N_CORES))],
                ins=[emb_stage[:, :]],
                outs=[emb_full[:, :]],
            )

            ct = cpool.tile([P, PPP * 3], U16)
            nc.sync.dma_start(out=ct[:], in_=coords_v)

            for l in range(NUM_LEVELS):
                R = int(RES[l])
                tab = tabp.tile([P, NE * FEAT], F16, tag="tab")
                nc.sync.dma_start(
                    out=tab[:],
                    in_=emb_full[l:l + 1, :].to_broadcast([P, NE * FEAT]),
                )
                for ti in range(NT):
                    co = ti * T * 3
                    cap = ct[:]
                    cviews = [
                        bass.AP(cap.tensor, cap.offset + co + a, [cap.ap[0], [3, T]])
                        for a in range(3)
                    ]

                    bf = bp.tile([P, NBF * T], F32, tag="bf")
                    bi = bp.tile([P, NBI * T], I32, tag="bi")

                    def fv(slot, dims=None, off=0):
                        return _ap(bf[:], 0, 1, P, slot * T + off, dims or [[1, T]])

                    def iv(slot, dims=None, off=0):
                        return _ap(bi[:], 0, 1, P, slot * T + off, dims or [[1, T]])

                    # dequant + floor + frac per axis
                    for a in range(3):
                        nc.vector.tensor_copy(out=fv(S_CU), in_=cviews[a])
                        nc.vector.tensor_scalar(out=fv(S_SX), in0=fv(S_CU),
                                                scalar1=float(R) / 65536.0,
                                                scalar2=None, op0=AL.mult)
                        nc.vector.tensor_copy(out=iv(S_XI), in_=fv(S_SX))
                        nc.vector.tensor_copy(out=fv(S_XF), in_=iv(S_XI))
                        nc.vector.tensor_tensor(out=fv(S_GT), in0=fv(S_XF), in1=fv(S_SX),
                                                op=AL.is_gt)
                        nc.vector.tensor_tensor(out=fv(S_FL + a), in0=fv(S_XF),
                                                in1=fv(S_GT), op=AL.subtract)
                        nc.vector.tensor_tensor(out=fv(S_FR + a), in0=fv(S_SX),
                                                in1=fv(S_FL + a), op=AL.subtract)
                        nc.vector.tensor_copy(out=iv(S_FI + a), in_=fv(S_FL + a))

                    if DENSE[l]:
                        Rp = R + 1
                        nc.vector.tensor_scalar(out=iv(S_HX1), in0=iv(S_FI + 1),
                                                scalar1=Rp, scalar2=None, op0=AL.mult)
                        nc.vector.tensor_tensor(out=iv(S_HY0), in0=iv(S_HX1),
                                                in1=iv(S_FI + 0), op=AL.add)
                        nc.vector.tensor_scalar(out=iv(S_HY1), in0=iv(S_FI + 2),
                                                scalar1=Rp * Rp, scalar2=None, op0=AL.mult)
                        nc.vector.tensor_tensor(out=iv(S_HZ0), in0=iv(S_HY0),
                                                in1=iv(S_HY1), op=AL.add)
                        for c in range(8):
                            i, j, k = (c >> 2) & 1, (c >> 1) & 1, c & 1
                            doff = i + Rp * j + Rp * Rp * k
                            ov = iv(S_IDX, [[8, T]], off=c)
                            nc.vector.tensor_scalar(out=ov, in0=iv(S_HZ0), scalar1=doff,
                                                    scalar2=None, op0=AL.add)
                    else:
                        nc.vector.tensor_scalar(out=iv(S_HX1), in0=iv(S_FI + 0),
                                                scalar1=1, scalar2=None, op0=AL.add)
                        for ax, pl, s0, s1 in ((1, P1L, S_HY0, S_HY1),
                                               (2, P2L, S_HZ0, S_HZ1)):
                            nc.vector.tensor_scalar(out=iv(S_TMP), in0=iv(S_FI + ax),
                                                    scalar1=pl, scalar2=None, op0=AL.mult)
                            nc.vector.tensor_scalar(out=iv(s0), in0=iv(S_TMP),
                                                    scalar1=NE - 1, scalar2=None,
                                                    op0=AL.bitwise_and)
                            nc.vector.tensor_scalar(out=iv(S_TMP), in0=iv(s0),
                                                    scalar1=pl, scalar2=None, op0=AL.add)
                            nc.vector.tensor_scalar(out=iv(s1), in0=iv(S_TMP),
                                                    scalar1=NE - 1, scalar2=None,
                                                    op0=AL.bitwise_and)
                        for i in range(2):
                            hxs = iv(S_FI + 0) if i == 0 else iv(S_HX1)
                            for j in range(2):
                                ov = iv(S_HXY, [[4, T]], off=i * 2 + j)
                                nc.vector.tensor_tensor(out=ov, in0=hxs,
                                                        in1=iv(S_HY0 if j == 0 else S_HY1),
                                                        op=AL.bitwise_xor)
                        for c in range(8):
                            i, j, k = (c >> 2) & 1, (c >> 1) & 1, c & 1
                            inv = iv(S_HXY, [[4, T]], off=i * 2 + j)
                            ov = iv(S_IDX, [[8, T]], off=c)
                            nc.vector.tensor_tensor(out=ov, in0=inv,
                                                    in1=iv(S_HZ0 if k == 0 else S_HZ1),
                                                    op=AL.bitwise_xor)

                    idx16 = xp.tile([P, T * 8], I16, tag="idx16")
                    nc.vector.tensor_copy(out=idx16[:],
                                          in_=iv(S_IDX, [[1, 8 * T]]))

                    # weights
                    for a in range(3):
                        nc.vector.tensor_scalar(out=fv(S_W0 + a), in0=fv(S_FR + a),
                                                scalar1=-1.0, scalar2=1.0,
                                                op0=AL.mult, op1=AL.add)
                    for i in range(2):
                        for j in range(2):
                            ov = fv(S_WXY, [[4, T]], off=i * 2 + j)
                            nc.vector.tensor_tensor(
                                out=ov, in0=fv(S_W0 + 0 if i == 0 else S_FR + 0),
                                in1=fv(S_W0 + 1 if j == 0 else S_FR + 1), op=AL.mult)
                    wt = xp.tile([P, T * 8], F32, tag="wt")
                    for c in range(8):
                        i, j, k = (c >> 2) & 1, (c >> 1) & 1, c & 1
                        inv = fv(S_WXY, [[4, T]], off=i * 2 + j)
                        ov = _ap(wt[:], 0, 1, P, c, [[8, T]])
                        nc.vector.tensor_tensor(out=ov, in0=inv,
                                                in1=fv(S_W0 + 2 if k == 0 else S_FR + 2),
                                                op=AL.mult)

                    gat = gpool.tile([P, NI * FEAT], F16, tag="gat")
                    nc.gpsimd.ap_gather(
                        out_ap=gat[:], in_ap=tab[:], idxs_ap=idx16[:],
                        channels=P, num_elems=NE, d=FEAT, num_idxs=NI,
                    )

                    # de-interleave: partition 16g+j's results live at slots
                    # s*16+j (replicated across the group); 16 partition-subset
                    # DMAs bring each partition its own (t,c,f)-ordered copy.
                    gx = xp.tile([P, T * 16], F16, tag="gx")
                    for j in range(16):
                        src = _ap(gat[:], j, 16, 8, j * 2, [[32, 8 * T], [1, 2]])
                        dst = _ap(gx[:], j, 16, 8, 0, [[1, 16 * T]])
                        nc.sync.dma_start(out=dst, in_=src)

                    res = iop.tile([P, T * FEAT], F32, tag="res")
                    gv = gx[:].rearrange("p (t c f) -> p t f c", c=8, f=2)
                    wv = _ap(wt[:], 0, 1, P, 0, [[8, T], [0, 2], [1, 8]])
                    pv = _ap(bf[:], 0, 1, P, S_PROD * T, [[16, T], [8, 2], [1, 8]])
                    nc.vector.tensor_tensor(out=pv, in0=gv, in1=wv, op=AL.mult)
                    pv2 = _ap(bf[:], 0, 1, P, S_PROD * T, [[16, T], [8, 2], [1, 8]])
                    rv = res[:].rearrange("p (t f) -> p t f", f=2)
                    nc.vector.tensor_reduce(out=rv, in_=pv2, axis=AX.X, op=AL.add)

                    row = (l * NT + ti) * P
                    nc.sync.dma_start(out=scratch[row:row + P, :], in_=res[:])

            for ti in range(NT):
                asm = iop.tile([P, T * 32], F32, tag="asm")
                for l in range(NUM_LEVELS):
                    slab = iop.tile([P, T * FEAT], F32, tag="slab")
                    row = (l * NT + ti) * P
                    nc.sync.dma_start(out=slab[:], in_=scratch[row:row + P, :])
                    av = asm[:].rearrange("p (t g) -> p t g", g=32)[:, :, 2 * l:2 * l + 2]
                    sv = slab[:].rearrange("p (t f) -> p t f", f=FEAT)
                    nc.vector.tensor_copy(out=av, in_=sv)

                # per-point scale = max |feat| over the 32 features
                m = iop.tile([P, 2 * T], F32, tag="m")
                mraw = _ap(m[:], 0, 1, P, 0, [[1, T]])
                mcl = _ap(m[:], 0, 1, P, T, [[1, T]])
                nc.vector.tensor_reduce(
                    out=mraw, in_=asm[:].rearrange("p (t g) -> p t g", g=32),
                    axis=AX.X, op=AL.max, apply_absolute_value=True)
                nc.vector.tensor_scalar(out=mcl, in0=mraw, scalar1=1e-30,
                                        scalar2=None, op0=AL.max)
                inv = iop.tile([P, 2 * T], F32, tag="inv")
                rcp = _ap(inv[:], 0, 1, P, 0, [[1, T]])
                rcp126 = _ap(inv[:], 0, 1, P, T, [[1, T]])
                nc.vector.reciprocal_approx_fast(out=rcp, in_=mcl)
                nc.vector.tensor_scalar(out=rcp126, in0=rcp, scalar1=QMAX,
                                        scalar2=None, op0=AL.mult)
                qf = iop.tile([P, T * 32], F32, tag="qf")
                nc.vector.tensor_tensor(
                    out=qf[:].rearrange("p (t g) -> p t g", g=32),
                    in0=asm[:].rearrange("p (t g) -> p t g", g=32),
                    in1=_ap(inv[:], 0, 1, P, T, [[1, T], [0, 32]]),
                    op=AL.mult)
                q8 = iop.tile([P, T * 32], I8, tag="q8")
                nc.vector.tensor_copy(out=q8[:], in_=qf[:])
                s16 = iop.tile([P, T], F16, tag="s16")
                nc.vector.tensor_copy(out=s16[:], in_=mcl)

                nc.sync.dma_start(
                    out=bass.AP(out_v.tensor, out_v.offset + ti * T * OB,
                                [out_v.ap[0], [OB, T], [1, 32]]),
                    in_=q8[:].rearrange("p (t g) -> p t g", g=32),
                )
                nc.sync.dma_start(
                    out=bass.AP(s_v.tensor, s_v.offset + ti * T * (OB // 2) + 16,
                                [s_v.ap[0], [OB // 2, T]]),
                    in_=s16[:],
                )
    nc.compile()
    return nc


class _Runner:
    """Builds the sharded jitted executable once; subsequent kernel() calls
    only pay input transfer + execute + output readback."""

    def __init__(self):
        import jax
        from concourse import bass2jax
        from jax.experimental.shard_map import shard_map
        from jax.sharding import Mesh, NamedSharding, PartitionSpec

        self.jax = jax
        bass2jax.install_neuronx_cc_hook()

        nc = _build_nc()
        partition_name = (
            nc.partition_id_tensor.name if nc.partition_id_tensor else None
        )

        in_names, out_names, out_avals = [], [], []
        for alloc in nc.m.functions[0].allocations:
            if not isinstance(alloc, mybir.MemoryLocationSet):
                continue
            name = alloc.memorylocations[0].name
            if alloc.kind == "ExternalInput":
                if name != partition_name:
                    in_names.append(name)
            elif alloc.kind == "ExternalOutput":
                out_names.append(name)
                out_avals.append(
                    jax.core.ShapedArray(
                        tuple(alloc.tensor_shape), mybir.dt.np(alloc.dtype)
                    )
                )
        n_params = len(in_names)
        n_outs = len(out_avals)
        in_names = in_names + out_names
        if partition_name is not None:
            in_names.append(partition_name)
        self.n_params = n_params
        self.out_names = out_names

        def _body(*args):
            operands = list(args)
            if partition_name is not None:
                operands.append(bass2jax.partition_id_tensor())
            outs = bass2jax._bass_exec_p.bind(
                *operands,
                out_avals=tuple(out_avals),
                in_names=tuple(in_names),
                out_names=tuple(out_names),
                lowering_input_output_aliases=(),
                sim_require_finite=True,
                sim_require_nnan=True,
                nc=nc,
            )
            return tuple(outs)

        devices = jax.devices()[:N_CORES]
        assert len(devices) == N_CORES
        mesh = Mesh(np.asarray(devices), ("core",))
        in_specs = (PartitionSpec("core"),) * (n_params + n_outs)
        out_specs = (PartitionSpec("core"),) * n_outs
        self.sharded = jax.jit(
            shard_map(_body, mesh=mesh, in_specs=in_specs,
                      out_specs=out_specs, check_rep=False),
            keep_unused=True,
        )
        # Persistent on-device dummies for the output operands: the kernel
        # writes every element of the output, so initial contents are
        # irrelevant; without donation these operands are never consumed.
        self.shd = NamedSharding(mesh, PartitionSpec("core"))
        self.out_dummies = [
            jax.jit(
                lambda av=av: jax.numpy.zeros(
                    (N_CORES * av.shape[0],) + tuple(av.shape[1:]), av.dtype
                ),
                out_shardings=self.shd,
            )()
            for av in out_avals
        ]


_RUNNER = None
_POOL = ThreadPoolExecutor(8)
_DPOOL = ThreadPoolExecutor(2)


def _get_runner():
    global _RUNNER
    if _RUNNER is None:
        _RUNNER = _Runner()
    return _RUNNER


def _decode_chunk(arr: np.ndarray, out: np.ndarray, row0: int) -> None:
    """arr: [CHUNK_N, 34] int8 -> out[row0:row0+CHUNK_N, :32] f32."""
    n = arr.shape[0]
    s = np.ascontiguousarray(arr[:, 32:34]).view(np.float16)
    su = s.astype(np.float32) * np.float32(1.0 / QMAX)  # [n,1]
    q = arr[:, :32]
    step = (n + 7) // 8

    def blk(i0):
        i1 = min(i0 + step, n)
        np.multiply(q[i0:i1].astype(np.float32), su[i0:i1],
                    out=out[row0 + i0:row0 + i1])

    list(_POOL.map(blk, range(0, n, step)))


def kernel(coords: np.ndarray, embeddings: np.ndarray) -> np.ndarray:
    coords = np.asarray(coords, dtype=np.float32)
    embeddings = np.asarray(embeddings, dtype=np.float32)

    cq = np.floor(coords * np.float32(65536.0)).astype(np.uint16)
    cq = np.ascontiguousarray(cq)

    emb16 = np.zeros((NUM_LEVELS, NE, FEAT), np.float16)
    for l in range(NUM_LEVELS):
        c = int(COUNTS[l])
        emb16[l, :c] = embeddings[int(OFFSETS[l]):int(OFFSETS[l]) + c].astype(np.float16)
    emb16 = emb16.reshape(NUM_LEVELS, NE * FEAT)

    r = _get_runner()
    out = np.empty((N_POINTS, 32), np.float32)

    # upload the table slices once; reuse the device array across chunks
    emb_dev = r.jax.device_put(emb16, r.shd)

    # dispatch all chunks (async), then fetch + decode pipelined
    chunk_outs = []
    for c in range(CHUNKS):
        cq_c = cq[c * CHUNK_N:(c + 1) * CHUNK_N]
        chunk_outs.append(r.sharded(cq_c, emb_dev, *r.out_dummies))

    futs = []
    for c in range(CHUNKS):
        arr = np.asarray(chunk_outs[c][0])          # blocking d2h fetch
        futs.append(_DPOOL.submit(_decode_chunk, arr, out, c * CHUNK_N))
    for f in futs:
        f.result()
    return out


# revision 11
# speedup vs baseline: 1.1629x; 1.1629x over previous
"""Instant-NGP hash-encoding forward on 8 TRN2 NeuronCores.

Data-parallel over points, processed in CHUNKS sequential launches per call
(pipelines device exec + host decode under the slow axon wire). Per core per
launch:
  - Each core receives 2 of the 16 level tables (fp16, padded to 16384
    entries); an AllGather over NeuronLink assembles the full table set in
    DRAM (cuts the host->device table upload 8x).
  - Per level the table is broadcast-DMA'd into SBUF across all 128
    partitions; gather via GPSIMD ap_gather; DVE computes cell coords,
    hash indices, trilinear weights, 8-corner weighted reduction.
  - Per-level results go to a DRAM scratch (level-major); a final pass
    interleaves them to [N, 32], quantizes to int8 with a per-point fp16
    scale (max |feat| over 32 features, reciprocal_approx_fast), and packs
    q[32]+scale[2B] into a 34-byte/point output row (single d2h tensor).

Wire-format rationale: the axon tunnel runs ~30-40MB/s half-duplex, so bytes
on the wire dominate wall time. coords ship as uint16 (floor(c*65536)),
output ships as int8+fp16-scale (34B/point); host decodes q * (scale/126).
The jitted shard_map executable is built ONCE and cached at module level.
"""
import numpy as np
from concurrent.futures import ThreadPoolExecutor

import concourse.bass as bass
import concourse.mybir as mybir
from concourse import bacc
from concourse.tile import TileContext

F32 = mybir.dt.float32
F16 = mybir.dt.float16
I32 = mybir.dt.int32
I16 = mybir.dt.int16
I8 = mybir.dt.int8
U16 = mybir.dt.uint16
AL = mybir.AluOpType
AX = mybir.AxisListType

NUM_LEVELS = 16
TABLE_SIZE = 2 ** 14
MIN_RES, MAX_RES = 16, 512
FEAT = 2
N_POINTS = 1 << 20
N_CORES = 8
PI1, PI2 = 2654435761, 805459861
P1L = PI1 & (TABLE_SIZE - 1)
P2L = PI2 & (TABLE_SIZE - 1)

_b = np.exp((np.log(MAX_RES) - np.log(MIN_RES)) / (NUM_LEVELS - 1))
RES = np.floor(MIN_RES * _b ** np.arange(NUM_LEVELS)).astype(np.int64)
COUNTS = np.minimum((RES + 1) ** 3, TABLE_SIZE)
OFFSETS = np.concatenate([[0], np.cumsum(COUNTS)])
DENSE = [int(COUNTS[l]) == int((RES[l] + 1) ** 3) for l in range(NUM_LEVELS)]

CHUNKS = 2                       # sequential launches per kernel() call
CHUNK_N = N_POINTS // CHUNKS     # global points per launch
NC_N = CHUNK_N // N_CORES        # 65536 points per core per launch
P = 128
PPP = NC_N // P                  # 512 points per partition
T = 64                           # points per partition per tile
NT = PPP // T                    # tiles per core
NI = 16 * T * 8                  # ap_gather num_idxs per 16-partition group
NE = TABLE_SIZE
LPC = NUM_LEVELS // N_CORES      # table levels shipped per core (allgathered)

QMAX = 126.0                     # int8 quant headroom (|q| <= 126)
OB = 34                          # output row bytes: 32 int8 feats + fp16 scale

# f32 blob slots (units of T elements)
S_SX, S_XF, S_GT = 0, 1, 2
S_FL = 3   # 3 slots
S_FR = 6   # 3 slots
S_W0 = 9   # 3 slots
S_WXY = 12  # 4 slots
S_PROD = 16  # 16 slots
S_CU = 32   # u16->f32 staging
NBF = 33
# i32 blob slots
S_XI = 0
S_FI = 1   # 3 slots
S_HX1 = 4
S_HY0, S_HY1, S_HZ0, S_HZ1 = 5, 6, 7, 8
S_TMP = 9
S_HXY = 10  # 4 slots
S_IDX = 14  # 8 slots
NBI = 22


def _ap(tile_ap, part_off, part_step, part_cnt, elem_off, dims):
    pitch = tile_ap.ap[0][0]
    return bass.AP(
        tile_ap.tensor,
        tile_ap.offset + part_off * pitch + elem_off,
        [[part_step * pitch, part_cnt]] + dims,
    )


def _build_nc():
    nc = bacc.Bacc("TRN2", target_bir_lowering=False, debug=False)
    coords = nc.dram_tensor("coords", [NC_N, 3], U16, kind="ExternalInput")
    emb_in = nc.dram_tensor("emb16", [LPC, NE * FEAT], F16, kind="ExternalInput")
    out = nc.dram_tensor("out", [NC_N, OB], I8, kind="ExternalOutput")
    out16 = out.bitcast(F16)      # [NC_N, OB//2]

    coords_v = coords[:, :].rearrange("(p q) c -> p (q c)", p=P)
    out_v = out[:, :].rearrange("(p q) g -> p (q g)", p=P)
    s_v = out16[:, :].rearrange("(p q) g -> p (q g)", p=P)

    with TileContext(nc) as tc:
        with tc.tile_pool(name="dram", bufs=1, space="DRAM") as dpool, \
             tc.tile_pool(name="tab", bufs=1) as tabp, \
             tc.tile_pool(name="coord", bufs=1) as cpool, \
             tc.tile_pool(name="gat", bufs=1) as gpool, \
             tc.tile_pool(name="blob", bufs=2) as bp, \
             tc.tile_pool(name="idxw", bufs=2) as xp, \
             tc.tile_pool(name="io", bufs=2) as iop:

            scratch = dpool.tile([NUM_LEVELS * NT * P, T * FEAT], F32)
            emb_stage = dpool.tile([LPC, NE * FEAT], F16)
            emb_full = dpool.tile([NUM_LEVELS, NE * FEAT], F16)

            nc.sync.dma_start(out=emb_stage[:, :], in_=emb_in[:, :])
            nc.gpsimd.collective_compute(
                kind="AllGather",
                op=AL.bypass,
                replica_groups=[list(range(N_CORES))],
                ins=[emb_stage[:, :]],
                outs=[emb_full[:, :]],
            )

            ct = cpool.tile([P, PPP * 3], U16)
            nc.sync.dma_start(out=ct[:], in_=coords_v)

            for l in range(NUM_LEVELS):
                R = int(RES[l])
                tab = tabp.tile([P, NE * FEAT], F16, tag="tab")
                nc.sync.dma_start(
                    out=tab[:],
                    in_=emb_full[l:l + 1, :].to_broadcast([P, NE * FEAT]),
                )
                for ti in range(NT):
                    co = ti * T * 3
                    cap = ct[:]
                    cviews = [
                        bass.AP(cap.tensor, cap.offset + co + a, [cap.ap[0], [3, T]])
                        for a in range(3)
                    ]

                    bf = bp.tile([P, NBF * T], F32, tag="bf")
                    bi = bp.tile([P, NBI * T], I32, tag="bi")

                    def fv(slot, dims=None, off=0):
                        return _ap(bf[:], 0, 1, P, slot * T + off, dims or [[1, T]])

                    def iv(slot, dims=None, off=0):
                        return _ap(bi[:], 0, 1, P, slot * T + off, dims or [[1, T]])

                    # dequant + floor + frac per axis
                    for a in range(3):
                        nc.vector.tensor_copy(out=fv(S_CU), in_=cviews[a])
                        nc.vector.tensor_scalar(out=fv(S_SX), in0=fv(S_CU),
                                                scalar1=float(R) / 65536.0,
                                                scalar2=None, op0=AL.mult)
                        nc.vector.tensor_copy(out=iv(S_XI), in_=fv(S_SX))
                        nc.vector.tensor_copy(out=fv(S_XF), in_=iv(S_XI))
                        nc.vector.tensor_tensor(out=fv(S_GT), in0=fv(S_XF), in1=fv(S_SX),
                                                op=AL.is_gt)
                        nc.vector.tensor_tensor(out=fv(S_FL + a), in0=fv(S_XF),
                                                in1=fv(S_GT), op=AL.subtract)
                        nc.vector.tensor_tensor(out=fv(S_FR + a), in0=fv(S_SX),
                                                in1=fv(S_FL + a), op=AL.subtract)
                        nc.vector.tensor_copy(out=iv(S_FI + a), in_=fv(S_FL + a))

                    if DENSE[l]:
                        Rp = R + 1
                        nc.vector.tensor_scalar(out=iv(S_HX1), in0=iv(S_FI + 1),
                                                scalar1=Rp, scalar2=None, op0=AL.mult)
                        nc.vector.tensor_tensor(out=iv(S_HY0), in0=iv(S_HX1),
                                                in1=iv(S_FI + 0), op=AL.add)
                        nc.vector.tensor_scalar(out=iv(S_HY1), in0=iv(S_FI + 2),
                                                scalar1=Rp * Rp, scalar2=None, op0=AL.mult)
                        nc.vector.tensor_tensor(out=iv(S_HZ0), in0=iv(S_HY0),
                                                in1=iv(S_HY1), op=AL.add)
                        for c in range(8):
                            i, j, k = (c >> 2) & 1, (c >> 1) & 1, c & 1
                            doff = i + Rp * j + Rp * Rp * k
                            ov = iv(S_IDX, [[8, T]], off=c)
                            nc.vector.tensor_scalar(out=ov, in0=iv(S_HZ0), scalar1=doff,
                                                    scalar2=None, op0=AL.add)
                    else:
                        nc.vector.tensor_scalar(out=iv(S_HX1), in0=iv(S_FI + 0),
                                                scalar1=1, scalar2=None, op0=AL.add)
                        for ax, pl, s0, s1 in ((1, P1L, S_HY0, S_HY1),
                                               (2, P2L, S_HZ0, S_HZ1)):
                            nc.vector.tensor_scalar(out=iv(S_TMP), in0=iv(S_FI + ax),
                                                    scalar1=pl, scalar2=None, op0=AL.mult)
                            nc.vector.tensor_scalar(out=iv(s0), in0=iv(S_TMP),
                                                    scalar1=NE - 1, scalar2=None,
                                                    op0=AL.bitwise_and)
                            nc.vector.tensor_scalar(out=iv(S_TMP), in0=iv(s0),
                                                    scalar1=pl, scalar2=None, op0=AL.add)
                            nc.vector.tensor_scalar(out=iv(s1), in0=iv(S_TMP),
                                                    scalar1=NE - 1, scalar2=None,
                                                    op0=AL.bitwise_and)
                        for i in range(2):
                            hxs = iv(S_FI + 0) if i == 0 else iv(S_HX1)
                            for j in range(2):
                                ov = iv(S_HXY, [[4, T]], off=i * 2 + j)
                                nc.vector.tensor_tensor(out=ov, in0=hxs,
                                                        in1=iv(S_HY0 if j == 0 else S_HY1),
                                                        op=AL.bitwise_xor)
                        for c in range(8):
                            i, j, k = (c >> 2) & 1, (c >> 1) & 1, c & 1
                            inv = iv(S_HXY, [[4, T]], off=i * 2 + j)
                            ov = iv(S_IDX, [[8, T]], off=c)
                            nc.vector.tensor_tensor(out=ov, in0=inv,
                                                    in1=iv(S_HZ0 if k == 0 else S_HZ1),
                                                    op=AL.bitwise_xor)

                    idx16 = xp.tile([P, T * 8], I16, tag="idx16")
                    nc.vector.tensor_copy(out=idx16[:],
                                          in_=iv(S_IDX, [[1, 8 * T]]))

                    # weights
                    for a in range(3):
                        nc.vector.tensor_scalar(out=fv(S_W0 + a), in0=fv(S_FR + a),
                                                scalar1=-1.0, scalar2=1.0,
                                                op0=AL.mult, op1=AL.add)
                    for i in range(2):
                        for j in range(2):
                            ov = fv(S_WXY, [[4, T]], off=i * 2 + j)
                            nc.vector.tensor_tensor(
                                out=ov, in0=fv(S_W0 + 0 if i == 0 else S_FR + 0),
                                in1=fv(S_W0 + 1 if j == 0 else S_FR + 1), op=AL.mult)
                    wt = xp.tile([P, T * 8], F32, tag="wt")
                    for c in range(8):
                        i, j, k = (c >> 2) & 1, (c >> 1) & 1, c & 1
                        inv = fv(S_WXY, [[4, T]], off=i * 2 + j)
                        ov = _ap(wt[:], 0, 1, P, c, [[8, T]])
                        nc.vector.tensor_tensor(out=ov, in0=inv,
                                                in1=fv(S_W0 + 2 if k == 0 else S_FR + 2),
                                                op=AL.mult)

                    gat = gpool.tile([P, NI * FEAT], F16, tag="gat")
                    nc.gpsimd.ap_gather(
                        out_ap=gat[:], in_ap=tab[:], idxs_ap=idx16[:],
                        channels=P, num_elems=NE, d=FEAT, num_idxs=NI,
                    )

                    # de-interleave: partition 16g+j's results live at slots
                    # s*16+j (replicated across the group); 16 partition-subset
                    # DMAs bring each partition its own (t,c,f)-ordered copy.
                    gx = xp.tile([P, T * 16], F16, tag="gx")
                    for j in range(16):
                        src = _ap(gat[:], j, 16, 8, j * 2, [[32, 8 * T], [1, 2]])
                        dst = _ap(gx[:], j, 16, 8, 0, [[1, 16 * T]])
                        nc.sync.dma_start(out=dst, in_=src)

                    res = iop.tile([P, T * FEAT], F32, tag="res")
                    gv = gx[:].rearrange("p (t c f) -> p t f c", c=8, f=2)
                    wv = _ap(wt[:], 0, 1, P, 0, [[8, T], [0, 2], [1, 8]])
                    pv = _ap(bf[:], 0, 1, P, S_PROD * T, [[16, T], [8, 2], [1, 8]])
                    nc.vector.tensor_tensor(out=pv, in0=gv, in1=wv, op=AL.mult)
                    pv2 = _ap(bf[:], 0, 1, P, S_PROD * T, [[16, T], [8, 2], [1, 8]])
                    rv = res[:].rearrange("p (t f) -> p t f", f=2)
                    nc.vector.tensor_reduce(out=rv, in_=pv2, axis=AX.X, op=AL.add)

                    row = (l * NT + ti) * P
                    nc.sync.dma_start(out=scratch[row:row + P, :], in_=res[:])

            for ti in range(NT):
                asm = iop.tile([P, T * 32], F32, tag="asm")
                for l in range(NUM_LEVELS):
                    slab = iop.tile([P, T * FEAT], F32, tag="slab")
                    row = (l * NT + ti) * P
                    nc.sync.dma_start(out=slab[:], in_=scratch[row:row + P, :])
                    av = asm[:].rearrange("p (t g) -> p t g", g=32)[:, :, 2 * l:2 * l + 2]
                    sv = slab[:].rearrange("p (t f) -> p t f", f=FEAT)
                    nc.vector.tensor_copy(out=av, in_=sv)

                # per-point scale = max |feat| over the 32 features
                m = iop.tile([P, 2 * T], F32, tag="m")
                mraw = _ap(m[:], 0, 1, P, 0, [[1, T]])
                mcl = _ap(m[:], 0, 1, P, T, [[1, T]])
                nc.vector.tensor_reduce(
                    out=mraw, in_=asm[:].rearrange("p (t g) -> p t g", g=32),
                    axis=AX.X, op=AL.max, apply_absolute_value=True)
                nc.vector.tensor_scalar(out=mcl, in0=mraw, scalar1=1e-30,
                                        scalar2=None, op0=AL.max)
                inv = iop.tile([P, 2 * T], F32, tag="inv")
                rcp = _ap(inv[:], 0, 1, P, 0, [[1, T]])
                rcp126 = _ap(inv[:], 0, 1, P, T, [[1, T]])
                nc.vector.reciprocal_approx_fast(out=rcp, in_=mcl)
                nc.vector.tensor_scalar(out=rcp126, in0=rcp, scalar1=QMAX,
                                        scalar2=None, op0=AL.mult)
                qf = iop.tile([P, T * 32], F32, tag="qf")
                nc.vector.tensor_tensor(
                    out=qf[:].rearrange("p (t g) -> p t g", g=32),
                    in0=asm[:].rearrange("p (t g) -> p t g", g=32),
                    in1=_ap(inv[:], 0, 1, P, T, [[1, T], [0, 32]]),
                    op=AL.mult)
                q8 = iop.tile([P, T * 32], I8, tag="q8")
                nc.vector.tensor_copy(out=q8[:], in_=qf[:])
                s16 = iop.tile([P, T], F16, tag="s16")
                nc.vector.tensor_copy(out=s16[:], in_=mcl)

                nc.sync.dma_start(
                    out=bass.AP(out_v.tensor, out_v.offset + ti * T * OB,
                                [out_v.ap[0], [OB, T], [1, 32]]),
                    in_=q8[:].rearrange("p (t g) -> p t g", g=32),
                )
                nc.sync.dma_start(
                    out=bass.AP(s_v.tensor, s_v.offset + ti * T * (OB // 2) + 16,
                                [s_v.ap[0], [OB // 2, T]]),
                    in_=s16[:],
                )
    nc.compile()
    return nc


class _Runner:
    """Builds the sharded jitted executable once; subsequent kernel() calls
    only pay input transfer + execute + output readback."""

    def __init__(self):
        import jax
        from concourse import bass2jax
        from jax.experimental.shard_map import shard_map
        from jax.sharding import Mesh, NamedSharding, PartitionSpec

        self.jax = jax
        bass2jax.install_neuronx_cc_hook()

        nc = _build_nc()
        partition_name = (
            nc.partition_id_tensor.name if nc.partition_id_tensor else None
        )

        in_names, out_names, out_avals = [], [], []
        for alloc in nc.m.functions[0].allocations:
            if not isinstance(alloc, mybir.MemoryLocationSet):
                continue
            name = alloc.memorylocations[0].name
            if alloc.kind == "ExternalInput":
                if name != partition_name:
                    in_names.append(name)
            elif alloc.kind == "ExternalOutput":
                out_names.append(name)
                out_avals.append(
                    jax.core.ShapedArray(
                        tuple(alloc.tensor_shape), mybir.dt.np(alloc.dtype)
                    )
                )
        n_params = len(in_names)
        n_outs = len(out_avals)
        in_names = in_names + out_names
        if partition_name is not None:
            in_names.append(partition_name)
        self.n_params = n_params
        self.out_names = out_names

        def _body(*args):
            operands = list(args)
            if partition_name is not None:
                operands.append(bass2jax.partition_id_tensor())
            outs = bass2jax._bass_exec_p.bind(
                *operands,
                out_avals=tuple(out_avals),
                in_names=tuple(in_names),
                out_names=tuple(out_names),
                lowering_input_output_aliases=(),
                sim_require_finite=True,
                sim_require_nnan=True,
                nc=nc,
            )
            return tuple(outs)

        devices = jax.devices()[:N_CORES]
        assert len(devices) == N_CORES
        mesh = Mesh(np.asarray(devices), ("core",))
        in_specs = (PartitionSpec("core"),) * (n_params + n_outs)
        out_specs = (PartitionSpec("core"),) * n_outs
        self.sharded = jax.jit(
            shard_map(_body, mesh=mesh, in_specs=in_specs,
                      out_specs=out_specs, check_rep=False),
            keep_unused=True,
        )
        # Persistent on-device dummies for the output operands: the kernel
        # writes every element of the output, so initial contents are
        # irrelevant; without donation these operands are never consumed.
        self.shd = NamedSharding(mesh, PartitionSpec("core"))
        self.out_dummies = [
            jax.jit(
                lambda av=av: jax.numpy.zeros(
                    (N_CORES * av.shape[0],) + tuple(av.shape[1:]), av.dtype
                ),
                out_shardings=self.shd,
            )()
            for av in out_avals
        ]


_RUNNER = None
_POOL = ThreadPoolExecutor(8)
_DPOOL = ThreadPoolExecutor(2)


def _get_runner():
    global _RUNNER
    if _RUNNER is None:
        _RUNNER = _Runner()
    return _RUNNER


def _decode_chunk(arr: np.ndarray, out: np.ndarray, row0: int) -> None:
    """arr: [CHUNK_N, 34] int8 -> out[row0:row0+CHUNK_N, :32] f32."""
    n = arr.shape[0]
    s = np.ascontiguousarray(arr[:, 32:34]).view(np.float16)
    su = s.astype(np.float32) * np.float32(1.0 / QMAX)  # [n,1]
    q = arr[:, :32]
    step = (n + 7) // 8

    def blk(i0):
        i1 = min(i0 + step, n)
        np.multiply(q[i0:i1], su[i0:i1], out=out[row0 + i0:row0 + i1],
                    casting="unsafe")

    list(_POOL.map(blk, range(0, n, step)))


def kernel(coords: np.ndarray, embeddings: np.ndarray) -> np.ndarray:
    coords = np.asarray(coords, dtype=np.float32)
    embeddings = np.asarray(embeddings, dtype=np.float32)

    cq = np.floor(coords * np.float32(65536.0)).astype(np.uint16)
    cq = np.ascontiguousarray(cq)

    emb16 = np.zeros((NUM_LEVELS, NE, FEAT), np.float16)
    for l in range(NUM_LEVELS):
        c = int(COUNTS[l])
        emb16[l, :c] = embeddings[int(OFFSETS[l]):int(OFFSETS[l]) + c].astype(np.float16)
    emb16 = emb16.reshape(NUM_LEVELS, NE * FEAT)

    r = _get_runner()
    out = np.empty((N_POINTS, 32), np.float32)

    # upload the table slices once; reuse the device array across chunks
    emb_dev = r.jax.device_put(emb16, r.shd)

    # dispatch all chunks (async), then fetch + decode pipelined
    chunk_outs = []
    for c in range(CHUNKS):
        cq_c = cq[c * CHUNK_N:(c + 1) * CHUNK_N]
        chunk_outs.append(r.sharded(cq_c, emb_dev, *r.out_dummies))

    futs = []
    for c in range(CHUNKS):
        arr = np.asarray(chunk_outs[c][0])          # blocking d2h fetch
        futs.append(_DPOOL.submit(_decode_chunk, arr, out, c * CHUNK_N))
    for f in futs:
        f.result()
    return out
